# revision 23
# baseline (speedup 1.0000x reference)
"""Trainium2 Bass kernel for MembranePotentialDecoder.

Computes the final state of the leaky-integrator scan
    mem_t = mem_{t-1} * decay + spike_t,  mem_{-1} = 0
which closed-form is the weighted reduction
    out[b, n] = sum_t decay^(T-1-t) * spikes[b, t, n].

Strategy: data-parallel over batch B across 8 NeuronCores (4 batches each,
16 MiB per core).  The reduction runs on the TensorEngine: matmuls with a
stationary per-partition weight column contract the 128 partitions,
accumulating a batch's row groups into PSUM per 512-wide column chunk, in
float32r (single-pass FP22 fp32, 1 cycle/row).  All loads stream on the sync
HWDGE ring; PSUM->SBUF copies split across DVE/ACT; stores go on the ACT
ring so the load ring never carries a semaphore-waiting instruction.

Laggard-engine derating: SDMA engine 15 (partitions 92-95/124-127) is
empirically ~20% slower on some cores and, since engine<->partition mapping
is fixed, sets the stream tail.  The weight column is per-partition DATA, so
rows can be assigned to any partition: those 8 partitions carry 3 rows per
batch instead of 4 (-25% bytes), and the 8 surplus rows ride a 5th
accumulation group on a rotating partition block, with zero weights padding
the inactive partitions (their SBUF is memset once so 0*x stays finite).
"""

import sys

import numpy as np

if "/opt/trn_rl_repo" not in sys.path:
    sys.path.insert(0, "/opt/trn_rl_repo")

import concourse.bass as bass  # noqa: F401  (engine namespaces live on nc)
import concourse.tile as tile
from concourse import bacc, mybir
from concourse.bass_utils import run_bass_kernel_spmd

TAU = 10.0
B, T, N = 32, 512, 2048
NCORES = 8
B_LOC = B // NCORES          # 4 batches per core
NCHUNK = N // 512            # 4 matmul column chunks (PSUM bank = 512 fp32)

# Row->partition assignment (same for every batch except the extras block):
# groups 0-2: row 128j+p on partition p (full tiles).
# group 3: rows 384..475 on partitions 0..91, rows 476..503 on partitions
#          96..123; engine-15 partitions {92-95, 124-127} carry nothing.
# group 4: rows 504..511 on an 8-partition block rotating per batch.
XTRA = [4, 12, 64, 72]       # start partition of the extras block, per batch

# Set by test harness to enable NTFF profiling; results stashed here.
PROFILE = False
LAST_RESULTS = None
_NC_CACHE = None


def _weights() -> np.ndarray:
    """(128, 8) fp32: cols 0-2 = groups 0-2, col 3 = group 3, cols 4+b =
    group 4 of batch b.  w = decay^(T-1-row), 0 on inactive partitions."""
    decay = np.float64(np.exp(np.float32(-1.0 / TAU), dtype=np.float32))

    def wrow(t):
        return decay ** (T - 1 - t)

    w = np.zeros((128, 8), dtype=np.float64)
    p = np.arange(128)
    for j in range(3):
        w[:, j] = wrow(128 * j + p)
    w[:92, 3] = wrow(384 + p[:92])
    w[96:124, 3] = wrow(476 + np.arange(28))
    for b in range(B_LOC):
        x0 = XTRA[b]
        w[x0 : x0 + 8, 4 + b] = wrow(504 + np.arange(8))
    return w.astype(np.float32)


def _build_program():
    nc = bacc.Bacc(
        "TRN2",
        target_bir_lowering=False,
        debug=False,
        enable_asserts=False,
        num_devices=NCORES,
    )
    f32 = mybir.dt.float32
    f32r = mybir.dt.float32r

    x = nc.dram_tensor("spikes", [B_LOC, T, N], f32r, kind="ExternalInput").ap()
    w = nc.dram_tensor("w", [128, 8], f32r, kind="ExternalInput").ap()
    out = nc.dram_tensor("out", [B_LOC, N], f32, kind="ExternalOutput").ap()

    with tile.TileContext(nc) as tc:
        with (
            tc.tile_pool(name="wpool", bufs=1) as wpool,
            tc.tile_pool(name="xpool", bufs=12) as xpool,
            tc.tile_pool(name="t3pool", bufs=1) as t3pool,
            tc.tile_pool(name="t4pool", bufs=1) as t4pool,
            tc.tile_pool(name="opool", bufs=2) as opool,
            tc.tile_pool(name="ppool", bufs=8, space="PSUM") as ppool,
        ):
            # tiny weight load via SWDGE so it never blocks the sync ring
            wt = wpool.tile([128, 8], f32r)
            nc.gpsimd.dma_start(wt[:], w[:])

            # double-buffered group-3/4 tiles, zero-filled once so the
            # inactive (zero-weight) partitions never feed NaN into 0*x
            t3s = [t3pool.tile([128, N], f32r, name=f"t3_{i}") for i in range(2)]
            t4s = [t4pool.tile([128, N], f32r, name=f"t4_{i}") for i in range(2)]
            for i in range(2):
                nc.gpsimd.memset(t3s[i][64:128, :].bitcast(f32), 0.0)
                nc.gpsimd.memset(t4s[i][:, :].bitcast(f32), 0.0)

            for b in range(B_LOC):
                pss = [
                    ppool.tile([1, 512], f32, name=f"ps{b}_{c}", tag="ps")
                    for c in range(NCHUNK)
                ]
                # groups 0-2: full contiguous 1 MiB tiles
                for j in range(3):
                    xt = xpool.tile([128, N], f32r, name="xt", tag="xt")
                    nc.sync.dma_start(xt[:], x[b, 128 * j : 128 * (j + 1)])
                    for c in range(NCHUNK):
                        nc.tensor.matmul(
                            pss[c][:],
                            wt[:, j : j + 1],
                            xt[:, c * 512 : (c + 1) * 512],
                            start=(j == 0),
                            stop=False,
                        )
                # group 4: 8 extras rows, small DMA issued before group 3's
                t4 = t4s[b % 2]
                x0 = XTRA[b]
                nc.sync.dma_start(t4[x0 : x0 + 8, :], x[b, 504:512])
                # group 3: 120 rows on the two non-derated partition ranges
                t3 = t3s[b % 2]
                nc.sync.dma_start(t3[0:92, :], x[b, 384:476])
                nc.sync.dma_start(t3[96:124, :], x[b, 476:504])
                for c in range(NCHUNK):
                    cs = slice(c * 512, (c + 1) * 512)
                    nc.tensor.matmul(
                        pss[c][:], wt[:, 4 + b : 5 + b], t4[:, cs],
                        start=False, stop=False,
                    )
                    nc.tensor.matmul(
                        pss[c][:], wt[:, 3:4], t3[:, cs],
                        start=False, stop=True,
                    )
                ot = opool.tile([1, N], f32)
                for c in range(NCHUNK):
                    # spread PSUM->SBUF copies across DVE and ACT
                    dst = ot[:, c * 512 : (c + 1) * 512]
                    if c % 2 == 0:
                        nc.vector.tensor_copy(dst, pss[c][:])
                    else:
                        nc.scalar.copy(dst, pss[c][:])
                # store on the ACT HWDGE ring: the sync ring must stay a
                # pure back-to-back input stream
                nc.scalar.dma_start(out[b : b + 1, :], ot[:])

    nc.compile()
    return nc


def kernel(spikes: np.ndarray) -> np.ndarray:
    global LAST_RESULTS, _NC_CACHE
    spikes = np.ascontiguousarray(np.asarray(spikes, dtype=np.float32))
    assert spikes.shape == (B, T, N), spikes.shape

    if _NC_CACHE is None:
        _NC_CACHE = _build_program()
    nc = _NC_CACHE
    w_in = _weights()
    in_maps = [
        {"spikes": spikes[i * B_LOC : (i + 1) * B_LOC], "w": w_in}
        for i in range(NCORES)
    ]
    res = run_bass_kernel_spmd(nc, in_maps, list(range(NCORES)), trace=PROFILE)
    LAST_RESULTS = res
    return np.concatenate([res.results[i]["out"] for i in range(NCORES)], axis=0)


# revision 26
# speedup vs baseline: 1.0026x; 1.0026x over previous
"""Trainium2 Bass kernel for MembranePotentialDecoder.

Computes the final state of the leaky-integrator scan
    mem_t = mem_{t-1} * decay + spike_t,  mem_{-1} = 0
which closed-form is the weighted reduction
    out[b, n] = sum_t decay^(T-1-t) * spikes[b, t, n].

Strategy: data-parallel over batch B across 8 NeuronCores (4 batches each,
16 MiB per core).  The reduction runs on the TensorEngine: matmuls with a
stationary per-partition weight column contract the 128 partitions,
accumulating a batch's row groups into PSUM per 512-wide column chunk, in
float32r (single-pass FP22 fp32, 1 cycle/row).  All loads stream on the sync
HWDGE ring; PSUM->SBUF copies split across DVE/ACT; stores go on the ACT
ring so the load ring never carries a semaphore-waiting instruction.

Laggard-engine derating: SDMA engine 15 (partitions 92-95/124-127) is
empirically ~20% slower on some cores and, since engine<->partition mapping
is fixed, sets the stream tail.  The weight column is per-partition DATA, so
rows can be assigned to any partition: those 8 partitions carry 3 rows per
batch instead of 4 (-25% bytes), and the 8 surplus rows ride a 5th
accumulation group on a rotating partition block, with zero weights padding
the inactive partitions (their SBUF is memset once so 0*x stays finite).
"""

import sys

import numpy as np

if "/opt/trn_rl_repo" not in sys.path:
    sys.path.insert(0, "/opt/trn_rl_repo")

import concourse.bass as bass  # noqa: F401  (engine namespaces live on nc)
import concourse.tile as tile
from concourse import bacc, mybir
from concourse.bass_utils import run_bass_kernel_spmd

TAU = 10.0
B, T, N = 32, 512, 2048
NCORES = 8
B_LOC = B // NCORES          # 4 batches per core
NCHUNK = N // 512            # 4 matmul column chunks (PSUM bank = 512 fp32)

# Row->partition assignment (same for every batch except the extras block):
# groups 0-2: row 128j+p on partition p (full tiles).
# group 3: rows 384..475 on partitions 0..91, rows 476..503 on partitions
#          96..123; engine-15 partitions {92-95, 124-127} carry nothing.
# group 4: rows 504..511 on an 8-partition block rotating per batch.
XTRA = [4, 12, 64, 72]       # start partition of the extras block, per batch

# Set by test harness to enable NTFF profiling; results stashed here.
PROFILE = False
LAST_RESULTS = None
_NC_CACHE = None


def _weights() -> np.ndarray:
    """(128, 8) fp32: cols 0-2 = groups 0-2, col 3 = group 3, cols 4+b =
    group 4 of batch b.  w = decay^(T-1-row), 0 on inactive partitions."""
    decay = np.float64(np.exp(np.float32(-1.0 / TAU), dtype=np.float32))

    def wrow(t):
        return decay ** (T - 1 - t)

    w = np.zeros((128, 8), dtype=np.float64)
    p = np.arange(128)
    for j in range(3):
        w[:, j] = wrow(128 * j + p)
    w[:92, 3] = wrow(384 + p[:92])
    w[96:124, 3] = wrow(476 + np.arange(28))
    for b in range(B_LOC):
        x0 = XTRA[b]
        w[x0 : x0 + 8, 4 + b] = wrow(504 + np.arange(8))
    return w.astype(np.float32)


def _build_program():
    nc = bacc.Bacc(
        "TRN2",
        target_bir_lowering=False,
        debug=False,
        enable_asserts=False,
        num_devices=NCORES,
    )
    f32 = mybir.dt.float32
    f32r = mybir.dt.float32r

    x = nc.dram_tensor("spikes", [B_LOC, T, N], f32r, kind="ExternalInput").ap()
    w = nc.dram_tensor("w", [128, 8], f32r, kind="ExternalInput").ap()
    out = nc.dram_tensor("out", [B_LOC, N], f32, kind="ExternalOutput").ap()

    with tile.TileContext(nc) as tc:
        with (
            tc.tile_pool(name="wpool", bufs=1) as wpool,
            tc.tile_pool(name="xpool", bufs=10) as xpool,
            tc.tile_pool(name="t3pool", bufs=1) as t3pool,
            tc.tile_pool(name="t4pool", bufs=1) as t4pool,
            tc.tile_pool(name="opool", bufs=2) as opool,
            tc.tile_pool(name="ppool", bufs=8, space="PSUM") as ppool,
        ):
            # tiny weight load via SWDGE so it never blocks the sync ring
            wt = wpool.tile([128, 8], f32r)
            nc.gpsimd.dma_start(wt[:], w[:])

            # one group-3/4 tile per batch (no reuse -> their DMAs carry no
            # WAR waits that would stall the sync ring), zero-filled once so
            # the inactive (zero-weight) partitions never feed NaN into 0*x;
            # memsets ordered by first need (t4 before t3, batch order)
            t3s = [t3pool.tile([128, N], f32r, name=f"t3_{i}") for i in range(B_LOC)]
            t4s = [t4pool.tile([128, N], f32r, name=f"t4_{i}") for i in range(B_LOC)]
            for i in range(B_LOC):
                nc.gpsimd.memset(t4s[i][:, :].bitcast(f32), 0.0)
                nc.gpsimd.memset(t3s[i][64:128, :].bitcast(f32), 0.0)

            for b in range(B_LOC):
                pss = [
                    ppool.tile([1, 512], f32, name=f"ps{b}_{c}", tag="ps")
                    for c in range(NCHUNK)
                ]
                # groups 0-2: full contiguous 1 MiB tiles
                for j in range(3):
                    xt = xpool.tile([128, N], f32r, name="xt", tag="xt")
                    nc.sync.dma_start(xt[:], x[b, 128 * j : 128 * (j + 1)])
                    for c in range(NCHUNK):
                        nc.tensor.matmul(
                            pss[c][:],
                            wt[:, j : j + 1],
                            xt[:, c * 512 : (c + 1) * 512],
                            start=(j == 0),
                            stop=False,
                        )
                # group 4: 8 extras rows, small DMA issued before group 3's
                t4 = t4s[b]
                x0 = XTRA[b]
                nc.sync.dma_start(t4[x0 : x0 + 8, :], x[b, 504:512])
                # group 3: 120 rows on the two non-derated partition ranges
                t3 = t3s[b]
                nc.sync.dma_start(t3[0:92, :], x[b, 384:476])
                nc.sync.dma_start(t3[96:124, :], x[b, 476:504])
                for c in range(NCHUNK):
                    cs = slice(c * 512, (c + 1) * 512)
                    nc.tensor.matmul(
                        pss[c][:], wt[:, 4 + b : 5 + b], t4[:, cs],
                        start=False, stop=False,
                    )
                    nc.tensor.matmul(
                        pss[c][:], wt[:, 3:4], t3[:, cs],
                        start=False, stop=True,
                    )
                ot = opool.tile([1, N], f32)
                for c in range(NCHUNK):
                    # spread PSUM->SBUF copies across DVE and ACT
                    dst = ot[:, c * 512 : (c + 1) * 512]
                    if c % 2 == 0:
                        nc.vector.tensor_copy(dst, pss[c][:])
                    else:
                        nc.scalar.copy(dst, pss[c][:])
                # store on the ACT HWDGE ring: the sync ring must stay a
                # pure back-to-back input stream
                nc.scalar.dma_start(out[b : b + 1, :], ot[:])

    nc.compile()
    return nc


def kernel(spikes: np.ndarray) -> np.ndarray:
    global LAST_RESULTS, _NC_CACHE
    spikes = np.ascontiguousarray(np.asarray(spikes, dtype=np.float32))
    assert spikes.shape == (B, T, N), spikes.shape

    if _NC_CACHE is None:
        _NC_CACHE = _build_program()
    nc = _NC_CACHE
    w_in = _weights()
    in_maps = [
        {"spikes": spikes[i * B_LOC : (i + 1) * B_LOC], "w": w_in}
        for i in range(NCORES)
    ]
    res = run_bass_kernel_spmd(nc, in_maps, list(range(NCORES)), trace=PROFILE)
    LAST_RESULTS = res
    return np.concatenate([res.results[i]["out"] for i in range(NCORES)], axis=0)


# revision 27
# speedup vs baseline: 1.4379x; 1.4341x over previous
"""Trainium2 Bass kernel for MembranePotentialDecoder.

Computes the final state of the leaky-integrator scan
    mem_t = mem_{t-1} * decay + spike_t,  mem_{-1} = 0
which closed-form is the weighted reduction
    out[b, n] = sum_t decay^(T-1-t) * spikes[b, t, n].

Strategy: data-parallel over batch B across 8 NeuronCores (4 batches each,
16 MiB per core).  Per core, each batch streams as four contiguous 1 MiB
t-tiles [128 partitions, 2048] (partition p = time row 128j+p), all 16 loads
issued unconditionally on the sync HWDGE ring (bufs=16, no slot waits) so the
input stream free-runs at the SDMA line rate (~25 GB/s/engine, ~405 GB/s
aggregate).  The weighted reduction over T runs on the TensorEngine: matmul
with the stationary weight column w[128j+p] contracts the 128 partitions,
accumulating the 4 t-tiles of a batch into PSUM per 512-wide column chunk.
float32r (single-pass FP22-truncated fp32 matmul) keeps the PE at 1
cycle/row so the kernel stays DMA-bound.  PSUM->SBUF copies split across
DVE/ACT; output stores go on the ACT HWDGE ring so the load ring never
carries a semaphore-waiting instruction; the final tile is loaded in four
512-column chunks so only one matmul+copy+store trails the last byte.
"""

import sys

import numpy as np

if "/opt/trn_rl_repo" not in sys.path:
    sys.path.insert(0, "/opt/trn_rl_repo")

import concourse.bass as bass  # noqa: F401  (engine namespaces live on nc)
import concourse.tile as tile
from concourse import bacc, mybir
from concourse.bass_utils import run_bass_kernel_spmd

TAU = 10.0
B, T, N = 32, 512, 2048
NCORES = 8
B_LOC = B // NCORES          # 4 batches per core
ROWS_PER_PART = T // 128     # 4 time rows folded into each partition
NCHUNK = N // 512            # 4 matmul column chunks (PSUM bank = 512 fp32)

# Set by test harness to enable NTFF profiling; results stashed here.
PROFILE = False
LAST_RESULTS = None
_NC_CACHE = None


def _weights() -> np.ndarray:
    """w_in[p, j] = decay^(T-1 - (128j + p)) as fp32: column j is the weight
    vector for t-tile j (rows 128j..128j+127 of the scan)."""
    decay = np.float64(np.exp(np.float32(-1.0 / TAU), dtype=np.float32))
    t = np.arange(128)[:, None] + 128 * np.arange(ROWS_PER_PART)[None, :]
    return (decay ** (T - 1 - t)).astype(np.float32)


def _build_program():
    nc = bacc.Bacc(
        "TRN2",
        target_bir_lowering=False,
        debug=False,
        enable_asserts=False,
        num_devices=NCORES,
    )
    f32 = mybir.dt.float32
    f32r = mybir.dt.float32r

    x = nc.dram_tensor("spikes", [B_LOC, T, N], f32r, kind="ExternalInput").ap()
    w = nc.dram_tensor("w", [128, ROWS_PER_PART], f32r, kind="ExternalInput").ap()
    out = nc.dram_tensor("out", [B_LOC, N], f32, kind="ExternalOutput").ap()

    with tile.TileContext(nc) as tc:
        with (
            tc.tile_pool(name="wpool", bufs=1) as wpool,
            tc.tile_pool(name="xpool", bufs=16) as xpool,
            tc.tile_pool(name="opool", bufs=2) as opool,
            tc.tile_pool(name="ppool", bufs=8, space="PSUM") as ppool,
        ):
            # tiny weight load goes via SWDGE so it never blocks the sync
            # HWDGE ring that streams the 1 MiB input tiles
            wt = wpool.tile([128, ROWS_PER_PART], f32r)
            nc.gpsimd.dma_start(wt[:], w[:])

            # x viewed as t-tiles: [b, j, p, n] with t = 128j + p
            xv = x.rearrange("b (j p) n -> b j p n", p=128)

            for b in range(B_LOC):
                pss = []
                for j in range(ROWS_PER_PART):
                    last_tile = b == B_LOC - 1 and j == ROWS_PER_PART - 1
                    ring = nc.sync
                    if last_tile:
                        # split the final tile into column chunks so only one
                        # matmul+copy+store trails the last byte of the stream
                        xt = xpool.tile([128, N], f32r, name="xt_last", tag="xt")
                        for c in range(NCHUNK):
                            cs = slice(c * 512, (c + 1) * 512)
                            ring.dma_start(xt[:, cs], xv[b, j][:, cs])
                            nc.tensor.matmul(
                                pss[c][:], wt[:, j : j + 1], xt[:, cs],
                                start=False, stop=True,
                            )
                        continue
                    xt = xpool.tile([128, N], f32r, name="xt", tag="xt")
                    ring.dma_start(xt[:], xv[b, j])
                    for c in range(NCHUNK):
                        if j == 0:
                            pss.append(
                                ppool.tile([1, 512], f32, name=f"ps{b}_{c}", tag="ps")
                            )
                        nc.tensor.matmul(
                            pss[c][:],
                            wt[:, j : j + 1],
                            xt[:, c * 512 : (c + 1) * 512],
                            start=(j == 0),
                            stop=(j == ROWS_PER_PART - 1),
                        )
                ot = opool.tile([1, N], f32)
                for c in range(NCHUNK):
                    # spread PSUM->SBUF copies across DVE and ACT
                    dst = ot[:, c * 512 : (c + 1) * 512]
                    if c % 2 == 0:
                        nc.vector.tensor_copy(dst, pss[c][:])
                    else:
                        nc.scalar.copy(dst, pss[c][:])
                # out DMA on the ACT HWDGE ring: the sync ring must stay a
                # pure back-to-back input stream (a sem-waiting out DMA on
                # it would stall all loads queued behind it)
                nc.scalar.dma_start(out[b : b + 1, :], ot[:])

    nc.compile()
    return nc


def kernel(spikes: np.ndarray) -> np.ndarray:
    global LAST_RESULTS, _NC_CACHE
    spikes = np.ascontiguousarray(np.asarray(spikes, dtype=np.float32))
    assert spikes.shape == (B, T, N), spikes.shape

    if _NC_CACHE is None:
        _NC_CACHE = _build_program()
    nc = _NC_CACHE
    w_in = _weights()
    in_maps = [
        {"spikes": spikes[i * B_LOC : (i + 1) * B_LOC], "w": w_in}
        for i in range(NCORES)
    ]
    res = run_bass_kernel_spmd(nc, in_maps, list(range(NCORES)), trace=PROFILE)
    LAST_RESULTS = res
    return np.concatenate([res.results[i]["out"] for i in range(NCORES)], axis=0)
